# revision 16
# baseline (speedup 1.0000x reference)
"""Multi-head causal attention (B=2, S=2048, D=1024, H=16) on 8 trn2 cores.

Sharding: core c -> batch b=c//4, head-group g=c%4 (heads 4g..4g+3).
Each core: Q/K/V projections for its heads from xT[b], causal attention in
transposed layout, row-parallel out-projection partial. Host sums the 4
partials per batch; partials ship as bf16 (halves D2H), bias is added on
device via a pre-broadcast bias tile during PSUM evacuation.

Schedule notes (v2):
- V projection runs in its own PSUM scope (closed before attention) so the
  attention phase gets scps x2 [2 banks each] + ctxps x2 + qk1ps x2 = 8 banks.
- Pair-1 Q/K projection chunks are interleaved into pair-0 attention to keep
  the PE dense while ACT chews exp (HAM stays warm).
- softmax normalize uses reciprocal_approx_fast (51 ULP, ~5x faster than
  vector.reciprocal).
- The second causal straddle (blocks d=2,3 of each q tile) narrows the d=3
  block to its live 128 columns: scores N=128, exp covers [2KB:QT+KB] only,
  PV for d=3 is N=128.
"""

import numpy as np

import concourse.bass as bass
import concourse.tile as tile
import concourse.mybir as mybir
from concourse import bacc
from concourse.bass_utils import run_bass_kernel_spmd

B, S, D, H, DH = 2, 2048, 1024, 16, 64
NCORES = 8
HPC = 4          # heads per core
PAIRS = 2        # head pairs per core
QT = 512         # q tile (free dim of scoresT / PV matmuls)
KB = 128         # k block (partition dim of scoresT)
NQT = S // QT    # 4
NKB = S // KB    # 16
DC = D // 128    # 8 contraction chunks for projections
SCALE = 1.0 / np.sqrt(DH)

F32 = mybir.dt.float32
BF = mybir.dt.bfloat16

ABLATE = set()
# bisect/devloop switches: "slow_recip", "no_straddle2", "no_biasadd"
VARIANTS = set()


def _build(reps=None):
    import contextlib
    nc = bacc.Bacc("TRN2", target_bir_lowering=False, debug=False, num_devices=NCORES)

    xT = nc.dram_tensor("xT", [D, S], BF, kind="ExternalInput").ap()
    wq = nc.dram_tensor("wq", [D, HPC * DH], BF, kind="ExternalInput").ap()
    wk = nc.dram_tensor("wk", [D, HPC * DH], BF, kind="ExternalInput").ap()
    wv = nc.dram_tensor("wv", [D, HPC * DH], BF, kind="ExternalInput").ap()
    wo = nc.dram_tensor("wo", [HPC * DH, D], BF, kind="ExternalInput").ap()
    bo_r = nc.dram_tensor("bo_r", [1, D], F32, kind="ExternalInput").ap()
    tri = nc.dram_tensor("tri", [KB, KB], BF, kind="ExternalInput").ap()
    out = nc.dram_tensor("out", [S, D], BF, kind="ExternalOutput").ap()

    with tile.TileContext(nc) as tc, \
         (tc.For_i(0, reps, 1) if reps else contextlib.nullcontext()), \
         tc.tile_pool(name="persist", bufs=1) as persist:
        # ---- persistent tiles ----
        qt_sb = [persist.tile([128, S], BF, name=f"qt{p}", tag=f"qt{p}") for p in range(PAIRS)]
        kt_sb = [persist.tile([128, S], BF, name=f"kt{p}", tag=f"kt{p}") for p in range(PAIRS)]
        # V' tiles: per s-block j, [128, 4*65]; head hl at cols 65*hl, ones col at 65*hl+64
        vt_sb = [persist.tile([128, HPC * (DH + 1)], BF, name=f"vt{j}", tag=f"vt{j}") for j in range(NKB)]
        ctx_sb = [persist.tile([128, S], BF, name=f"ctx{p}", tag=f"ctx{p}") for p in range(PAIRS)]
        wo_sb = [persist.tile([128, D], BF, name=f"wo{p}", tag=f"wo{p}") for p in range(PAIRS)]
        tri_sb = persist.tile([KB, KB], BF, name="tri", tag="tri")
        bo_sb = persist.tile([1, D], F32, name="bo", tag="bo")
        bo_bc = persist.tile([128, D], F32, name="bo_bc", tag="bo_bc")

        xts = [persist.tile([128, S], BF, name=f"xts{i}", tag=f"xts{i}") for i in range(DC)]
        wq_sb = [persist.tile([128, HPC * DH], BF, name=f"wq{i}", tag=f"wq{i}") for i in range(DC)]
        wk_sb = [persist.tile([128, HPC * DH], BF, name=f"wk{i}", tag=f"wk{i}") for i in range(DC)]
        wv_sb = [persist.tile([128, HPC * DH], BF, name=f"wv{i}", tag=f"wv{i}") for i in range(DC)]

        nc.sync.dma_start(tri_sb[:], tri[:])
        nc.sync.dma_start(bo_sb[:], bo_r[:])
        nc.gpsimd.partition_broadcast(bo_bc[:], bo_sb[:])
        for i in range(DC):
            nc.sync.dma_start(xts[i][:], xT[i * 128:(i + 1) * 128, :])
            nc.sync.dma_start(wq_sb[i][:], wq[i * 128:(i + 1) * 128, :])
            nc.sync.dma_start(wk_sb[i][:], wk[i * 128:(i + 1) * 128, :])
            nc.sync.dma_start(wv_sb[i][:], wv[i * 128:(i + 1) * 128, :])
        for p in range(PAIRS):
            nc.sync.dma_start(wo_sb[p][:], wo[p * 128:(p + 1) * 128, :])

        def proj_qk_chunked(p, pool):
            """q/k projection for pair p, D-chunk-outer so matmuls chase the
            xT DMAs chunk by chunk. Holds 8 psum banks. ACT/DVE evac split
            (ACT is idle in this phase)."""
            qps = [pool.tile([128, QT], F32, name=f"qps{st}", tag=f"qk{st}") for st in range(NQT)]
            kps = [pool.tile([128, QT], F32, name=f"kps{st}", tag=f"qk{4 + st}") for st in range(NQT)]
            for i in range(DC):
                for st in range(NQT):
                    nc.tensor.matmul(
                        qps[st][:], wq_sb[i][:, p * 128:(p + 1) * 128],
                        xts[i][:, st * QT:(st + 1) * QT],
                        start=(i == 0), stop=(i == DC - 1))
                for st in range(NQT):
                    nc.tensor.matmul(
                        kps[st][:], wk_sb[i][:, p * 128:(p + 1) * 128],
                        xts[i][:, st * QT:(st + 1) * QT],
                        start=(i == 0), stop=(i == DC - 1))
            for st in range(NQT):
                nc.scalar.copy(qt_sb[p][:, st * QT:(st + 1) * QT], qps[st][:])
                nc.vector.tensor_copy(kt_sb[p][:, st * QT:(st + 1) * QT], kps[st][:])

        def proj_qk_chunk(p, st, pool):
            """One q + one k projection chunk for pair p, s-tile st (DVE evac:
            runs interleaved with attention where ACT is exp-bound)."""
            qp = pool.tile([128, QT], F32, name="qp", tag="qkseq")
            for i in range(DC):
                nc.tensor.matmul(
                    qp[:], wq_sb[i][:, p * 128:(p + 1) * 128],
                    xts[i][:, st * QT:(st + 1) * QT],
                    start=(i == 0), stop=(i == DC - 1))
            nc.vector.tensor_copy(qt_sb[p][:, st * QT:(st + 1) * QT], qp[:])
            kp = pool.tile([128, QT], F32, name="kp", tag="qkseq")
            for i in range(DC):
                nc.tensor.matmul(
                    kp[:], wk_sb[i][:, p * 128:(p + 1) * 128],
                    xts[i][:, st * QT:(st + 1) * QT],
                    start=(i == 0), stop=(i == DC - 1))
            nc.vector.tensor_copy(kt_sb[p][:, st * QT:(st + 1) * QT], kp[:])

        def attention(p, h, qt_i, scps, ctxps, att, attsm):
            """scores->exp->mask->PV for one (pair, head, q-tile). Returns the
            cps PSUM tile (rows 0:DH = unnormalized ctx, row DH = denom l);
            the caller normalizes (batched across both heads of the pair)."""
            hl = 2 * p + h
            r0, r1 = h * 64, h * 64 + 64
            q0 = qt_i * QT
            nkb = 4 * (qt_i + 1)
            cps = ctxps.tile([DH + 1, QT], F32, name="cps", tag="cps")
            for g0 in range(0, nkb, 2):
                sp = scps.tile([128, 2 * QT], F32, name="sp", tag="sp")
                straddle2 = (g0 == 4 * qt_i + 2) and "masks" not in ABLATE \
                    and "no_straddle2" not in VARIANTS
                if straddle2:
                    # d=2 block full width; d=3 block only its live+diag 128
                    # cols, packed at sp[:, QT:QT+KB]
                    nc.tensor.matmul(
                        sp[:, 0:QT],
                        kt_sb[p][r0:r1, g0 * KB:(g0 + 1) * KB],
                        qt_sb[p][r0:r1, q0:q0 + QT],
                        start=True, stop=True)
                    nc.tensor.matmul(
                        sp[:, QT:QT + KB],
                        kt_sb[p][r0:r1, (g0 + 1) * KB:(g0 + 2) * KB],
                        qt_sb[p][r0:r1, q0 + 3 * KB:q0 + 4 * KB],
                        start=True, stop=True)
                else:
                    for u in range(2):
                        kb = g0 + u
                        nc.tensor.matmul(
                            sp[:, u * QT:(u + 1) * QT],
                            kt_sb[p][r0:r1, kb * KB:(kb + 1) * KB],
                            qt_sb[p][r0:r1, q0:q0 + QT],
                            start=True, stop=True)
                pt = att.tile([128, 2 * QT], BF, name="pt", tag="pt")
                if straddle2:
                    # exp only the live region [2KB:QT] of d=2 + packed d=3
                    nc.scalar.activation(
                        pt[:, 2 * KB:QT + KB], sp[:, 2 * KB:QT + KB],
                        mybir.ActivationFunctionType.Exp, scale=float(SCALE))
                    nc.gpsimd.memset(pt[:, 0:2 * KB], 0.0)
                    for off in (2 * KB, QT):
                        nc.vector.tensor_mul(
                            pt[:, off:off + KB], pt[:, off:off + KB], tri_sb[:])
                else:
                    nc.scalar.activation(
                        pt[:], sp[:], mybir.ActivationFunctionType.Exp,
                        scale=float(SCALE))
                    # causal masking for the d=0,1 straddle: memset the dead
                    # rectangle, multiply the [128,128] diagonal triangles
                    if "masks" in ABLATE:
                        pass
                    elif g0 == 4 * qt_i:          # blocks d=0, d=1
                        nc.gpsimd.memset(pt[:, QT:QT + KB], 0.0)
                        for off in (0, QT + KB):
                            nc.vector.tensor_mul(
                                pt[:, off:off + KB], pt[:, off:off + KB], tri_sb[:])
                    elif g0 == 4 * qt_i + 2:      # no_straddle2 fallback
                        nc.gpsimd.memset(pt[:, 0:2 * KB], 0.0)
                        nc.gpsimd.memset(pt[:, QT:QT + 3 * KB], 0.0)
                        for off in (2 * KB, QT + 3 * KB):
                            nc.vector.tensor_mul(
                                pt[:, off:off + KB], pt[:, off:off + KB], tri_sb[:])
                if straddle2:
                    # d=3 narrow PV first (no stop), then d=2 full width with
                    # stop=True so the accumulation group closes full-width
                    nc.tensor.matmul(
                        cps[:, 3 * KB:QT],
                        vt_sb[g0 + 1][:, hl * (DH + 1):(hl + 1) * (DH + 1)],
                        pt[:, QT:QT + KB],
                        start=False, stop=False)
                    nc.tensor.matmul(
                        cps[:],
                        vt_sb[g0][:, hl * (DH + 1):(hl + 1) * (DH + 1)],
                        pt[:, 0:QT],
                        start=(g0 == 0), stop=(g0 + 1 == nkb - 1))
                else:
                    for u in range(2):
                        kb = g0 + u
                        nc.tensor.matmul(
                            cps[:],
                            vt_sb[kb][:, hl * (DH + 1):(hl + 1) * (DH + 1)],
                            pt[:, u * QT:(u + 1) * QT],
                            start=(kb == 0), stop=(kb == nkb - 1))
            return cps

        def normalize_pair(p, qt_i, cps_by_h, attsm):
            """Batched softmax normalize for both heads of (p, qt): one
            [2,QT] reciprocal (costs the same as [1,QT]), then per-head
            broadcast + ctx multiply."""
            q0 = qt_i * QT
            # head rows live at partitions 0 and 32 (SBUF APs must start at
            # 32-aligned partitions); one reciprocal over [33,QT] costs the
            # same as [1,QT] (DVE time is free-dim-bound)
            r2 = attsm.tile([33, QT], F32, name="r2", tag="r2")
            for h in range(2):
                nc.vector.tensor_copy(
                    r2[32 * h:32 * h + 1, :], cps_by_h[h][DH:DH + 1, :])
            rr = attsm.tile([33, QT], F32, name="rr", tag="rr")
            if "recip" in ABLATE:
                nc.vector.tensor_copy(rr[:], r2[:])
            else:
                nc.vector.reciprocal(rr[:], r2[:])
            # partition_broadcast only reads a tile-base row: h=1's recip row
            # hops through a fresh tile via a gpsimd (1-input, no DVE-port
            # contention) copy first
            r1row = attsm.tile([1, QT], F32, name="r1row", tag="r1row")
            nc.gpsimd.tensor_copy(r1row[:], rr[32:33, :])
            for h in range(2):
                r0, r1 = h * 64, h * 64 + 64
                rb = attsm.tile([64, QT], F32, name="rb", tag="rb")
                nc.gpsimd.partition_broadcast(
                    rb[:], rr[0:1, :] if h == 0 else r1row[:])
                nc.vector.tensor_mul(
                    ctx_sb[p][r0:r1, q0:q0 + QT], cps_by_h[h][0:DH, :], rb[:])

        def outproj(qt_i, ph3ps, ph3sb):
            """partial out-projection rows for one q tile; bias added during
            the PSUM->SBUF evacuation (DVE), output bf16, then DMA out."""
            for qb in range(qt_i * 4, qt_i * 4 + 4):
                os_ = ph3sb.tile([128, D], BF, name="os", tag="os")
                for nh in range(2):
                    op = ph3ps.tile([128, 512], F32, name="op", tag="op")
                    for p in range(PAIRS):
                        nc.tensor.matmul(
                            op[:], ctx_sb[p][:, qb * 128:(qb + 1) * 128],
                            wo_sb[p][:, nh * 512:(nh + 1) * 512],
                            start=(p == 0), stop=(p == PAIRS - 1))
                    if "outio" in ABLATE:
                        continue
                    if "no_biasadd" in VARIANTS:
                        nc.vector.tensor_copy(os_[:, nh * 512:(nh + 1) * 512], op[:])
                    else:
                        nc.vector.tensor_add(
                            os_[:, nh * 512:(nh + 1) * 512], op[:],
                            bo_bc[:, nh * 512:(nh + 1) * 512])
                if "outdma" not in ABLATE and "outio" not in ABLATE:
                    nc.sync.dma_start(out[qb * 128:(qb + 1) * 128, :], os_[:])

        # phase A: q/k pair 0, chunk-pipelined against the input DMAs
        with tc.tile_pool(name="qk0ps", bufs=1, space="PSUM") as qk0ps:
            proj_qk_chunked(0, qk0ps)

        # phase B: V projection (own scope, closes before attention)
        skip_attn = "attn" in ABLATE
        with tc.tile_pool(name="vps", bufs=2, space="PSUM") as vps:
            for j in range(NKB):
                vp = vps.tile([128, HPC * DH], F32, name="vp", tag="vp")
                for i in range(DC):
                    nc.tensor.matmul(
                        vp[:], xts[i][:, j * 128:(j + 1) * 128], wv_sb[i][:],
                        start=(i == 0), stop=(i == DC - 1))
                vt_view = vt_sb[j].rearrange("p (h e) -> p h e", h=HPC)
                nc.vector.tensor_copy(
                    vt_view[:, :, 0:DH], vp.rearrange("p (h e) -> p h e", h=HPC))
                nc.gpsimd.memset(vt_view[:, :, DH:DH + 1], 1.0)

        # phase C: pair-0 attention with pair-1 q/k proj chunks interleaved
        with tc.tile_pool(name="att", bufs=6) as att, \
             tc.tile_pool(name="attsm", bufs=4) as attsm:

            with tc.tile_pool(name="scpsC", bufs=2, space="PSUM") as scpsC, \
                 tc.tile_pool(name="ctxpsC", bufs=3, space="PSUM") as ctxpsC, \
                 tc.tile_pool(name="qk1ps", bufs=1, space="PSUM") as qk1ps:
                for qt_i in range(NQT):
                    if not skip_attn:
                        cps_by_h = [attention(0, h, qt_i, scpsC, ctxpsC, att, attsm)
                                    for h in range(2)]
                        normalize_pair(0, qt_i, cps_by_h, attsm)
                    proj_qk_chunk(1, qt_i, qk1ps)

            # phase D: pair-1 attention, out-projection interleaved per
            # finished q tile
            with tc.tile_pool(name="scpsD", bufs=2, space="PSUM") as scpsD, \
                 tc.tile_pool(name="ctxpsD", bufs=2, space="PSUM") as ctxpsD, \
                 tc.tile_pool(name="ph3ps", bufs=2, space="PSUM") as ph3ps, \
                 tc.tile_pool(name="ph3sb", bufs=3) as ph3sb:
                for qt_i in range(NQT):
                    if not skip_attn:
                        cps_by_h = [attention(1, h, qt_i, scpsD, ctxpsD, att, attsm)
                                    for h in range(2)]
                        normalize_pair(1, qt_i, cps_by_h, attsm)
                    if "outproj" not in ABLATE and not skip_attn and qt_i > 0:
                        outproj(qt_i - 1, ph3ps, ph3sb)
                if "outproj" not in ABLATE and not skip_attn:
                    outproj(NQT - 1, ph3ps, ph3sb)

    nc.compile()
    return nc


_NC = None
PROFILE = False
TRACE_CORES = (0,)
LAST_RESULT = None


def _get_nc():
    global _NC
    if _NC is None:
        _NC = _build()
    return _NC


def kernel(x, Wq, Wk, Wv, Wo, bo):
    x = np.asarray(x, dtype=np.float32)
    Wq = np.asarray(Wq, dtype=np.float32)
    Wk = np.asarray(Wk, dtype=np.float32)
    Wv = np.asarray(Wv, dtype=np.float32)
    Wo = np.asarray(Wo, dtype=np.float32)
    bo = np.asarray(bo, dtype=np.float32)

    nc = _get_nc()

    in_maps = _prepare_in_maps(x, Wq, Wk, Wv, Wo, bo)

    global LAST_RESULT
    kw = {}
    if PROFILE:
        kw = dict(trace=True, trace_cores=list(TRACE_CORES))
    res = run_bass_kernel_spmd(nc, in_maps, core_ids=list(range(NCORES)), **kw)
    LAST_RESULT = res

    out = np.zeros((B, S, D), np.float32)
    for c in range(NCORES):
        b = c // 4
        out[b] += res.results[c]["out"].astype(np.float32)
    return out


def _prepare_in_maps(x, Wq, Wk, Wv, Wo, bo):
    kk = np.arange(KB)[:, None]
    qq = np.arange(KB)[None, :]
    import ml_dtypes
    tri = (kk <= qq).astype(ml_dtypes.bfloat16)

    bf16 = ml_dtypes.bfloat16
    xTs = [np.ascontiguousarray(x[b].T).astype(bf16) for b in range(B)]
    bo_row = np.ascontiguousarray(bo[None, :]).astype(np.float32)
    zeros_row = np.zeros((1, D), np.float32)

    in_maps = []
    for c in range(NCORES):
        b, g = divmod(c, 4)
        cs = slice(g * HPC * DH, (g + 1) * HPC * DH)
        in_maps.append({
            "xT": xTs[b],
            "wq": np.ascontiguousarray(Wq[:, cs]).astype(bf16),
            "wk": np.ascontiguousarray(Wk[:, cs]).astype(bf16),
            "wv": np.ascontiguousarray(Wv[:, cs]).astype(bf16),
            "wo": np.ascontiguousarray(Wo[cs, :]).astype(bf16),
            "bo_r": bo_row if g == 0 else zeros_row,
            "tri": tri,
        })
    return in_maps


# revision 21
# speedup vs baseline: 1.5372x; 1.5372x over previous
"""Multi-head causal attention (B=2, S=2048, D=1024, H=16) on 8 trn2 cores.

Sharding: core c -> batch b=c//4, head-group g=c%4 (heads 4g..4g+3).
Each core: Q/K/V projections for its heads from xT[b], causal attention in
transposed layout, row-parallel out-projection partial. Host sums the 4
partials per batch; partials ship as bf16 (halves D2H), bias is added on
device via a pre-broadcast bias tile during PSUM evacuation.

Schedule notes (v2):
- V projection runs in its own PSUM scope (closed before attention) so the
  attention phase gets scps x2 [2 banks each] + ctxps x2 + qk1ps x2 = 8 banks.
- Pair-1 Q/K projection chunks are interleaved into pair-0 attention to keep
  the PE dense while ACT chews exp (HAM stays warm).
- softmax normalize uses reciprocal_approx_fast (51 ULP, ~5x faster than
  vector.reciprocal).
- The second causal straddle (blocks d=2,3 of each q tile) narrows the d=3
  block to its live 128 columns: scores N=128, exp covers [2KB:QT+KB] only,
  PV for d=3 is N=128.
"""

import numpy as np

import concourse.bass as bass
import concourse.tile as tile
import concourse.mybir as mybir
from concourse import bacc
from concourse.bass_utils import run_bass_kernel_spmd

B, S, D, H, DH = 2, 2048, 1024, 16, 64
NCORES = 8
HPC = 4          # heads per core
PAIRS = 2        # head pairs per core
QT = 512         # q tile (free dim of scoresT / PV matmuls)
KB = 128         # k block (partition dim of scoresT)
NQT = S // QT    # 4
NKB = S // KB    # 16
DC = D // 128    # 8 contraction chunks for projections
SCALE = 1.0 / np.sqrt(DH)

F32 = mybir.dt.float32
BF = mybir.dt.bfloat16

ABLATE = set()
# bisect/devloop switches: "slow_recip", "no_straddle2", "no_biasadd"
VARIANTS = set()


def _build(reps=None):
    import contextlib
    nc = bacc.Bacc("TRN2", target_bir_lowering=False, debug=False, num_devices=NCORES)

    xT = nc.dram_tensor("xT", [D, S], BF, kind="ExternalInput").ap()
    wq = nc.dram_tensor("wq", [D, HPC * DH], BF, kind="ExternalInput").ap()
    wk = nc.dram_tensor("wk", [D, HPC * DH], BF, kind="ExternalInput").ap()
    wv = nc.dram_tensor("wv", [D, HPC * DH], BF, kind="ExternalInput").ap()
    wo = nc.dram_tensor("wo", [HPC * DH, D], BF, kind="ExternalInput").ap()
    bo_r = nc.dram_tensor("bo_r", [1, D], F32, kind="ExternalInput").ap()
    tri = nc.dram_tensor("tri", [KB, KB], BF, kind="ExternalInput").ap()
    out = nc.dram_tensor("out", [S, D], BF, kind="ExternalOutput").ap()

    with tile.TileContext(nc) as tc, \
         (tc.For_i(0, reps, 1) if reps else contextlib.nullcontext()), \
         tc.tile_pool(name="persist", bufs=1) as persist:
        # ---- persistent tiles ----
        qt_sb = [persist.tile([128, S], BF, name=f"qt{p}", tag=f"qt{p}") for p in range(PAIRS)]
        kt_sb = [persist.tile([128, S], BF, name=f"kt{p}", tag=f"kt{p}") for p in range(PAIRS)]
        # V' tiles: per s-block j, [128, 4*65]; head hl at cols 65*hl, ones col at 65*hl+64
        vt_sb = [persist.tile([128, HPC * (DH + 1)], BF, name=f"vt{j}", tag=f"vt{j}") for j in range(NKB)]
        ctx_sb = [persist.tile([128, S], BF, name=f"ctx{p}", tag=f"ctx{p}") for p in range(PAIRS)]
        wo_sb = [persist.tile([128, D], BF, name=f"wo{p}", tag=f"wo{p}") for p in range(PAIRS)]
        tri_sb = persist.tile([KB, KB], BF, name="tri", tag="tri")
        bo_sb = persist.tile([1, D], F32, name="bo", tag="bo")
        bo_bc = persist.tile([128, D], F32, name="bo_bc", tag="bo_bc")

        xts = [persist.tile([128, S], BF, name=f"xts{i}", tag=f"xts{i}") for i in range(DC)]
        wq_sb = [persist.tile([128, HPC * DH], BF, name=f"wq{i}", tag=f"wq{i}") for i in range(DC)]
        wk_sb = [persist.tile([128, HPC * DH], BF, name=f"wk{i}", tag=f"wk{i}") for i in range(DC)]
        wv_sb = [persist.tile([128, HPC * DH], BF, name=f"wv{i}", tag=f"wv{i}") for i in range(DC)]

        nc.sync.dma_start(tri_sb[:], tri[:])
        nc.sync.dma_start(bo_sb[:], bo_r[:])
        nc.gpsimd.partition_broadcast(bo_bc[:], bo_sb[:])
        for i in range(DC):
            nc.sync.dma_start(xts[i][:], xT[i * 128:(i + 1) * 128, :])
            nc.sync.dma_start(wq_sb[i][:], wq[i * 128:(i + 1) * 128, :])
            nc.sync.dma_start(wk_sb[i][:], wk[i * 128:(i + 1) * 128, :])
            nc.sync.dma_start(wv_sb[i][:], wv[i * 128:(i + 1) * 128, :])
        for p in range(PAIRS):
            nc.sync.dma_start(wo_sb[p][:], wo[p * 128:(p + 1) * 128, :])

        def proj_qk_chunked(p, pool):
            """q/k projection for pair p, D-chunk-outer so matmuls chase the
            xT DMAs chunk by chunk. Holds 8 psum banks. ACT/DVE evac split
            (ACT is idle in this phase)."""
            qps = [pool.tile([128, QT], F32, name=f"qps{st}", tag=f"qk{st}") for st in range(NQT)]
            kps = [pool.tile([128, QT], F32, name=f"kps{st}", tag=f"qk{4 + st}") for st in range(NQT)]
            for i in range(DC):
                for st in range(NQT):
                    nc.tensor.matmul(
                        qps[st][:], wq_sb[i][:, p * 128:(p + 1) * 128],
                        xts[i][:, st * QT:(st + 1) * QT],
                        start=(i == 0), stop=(i == DC - 1))
                for st in range(NQT):
                    nc.tensor.matmul(
                        kps[st][:], wk_sb[i][:, p * 128:(p + 1) * 128],
                        xts[i][:, st * QT:(st + 1) * QT],
                        start=(i == 0), stop=(i == DC - 1))
            for st in range(NQT):
                nc.scalar.copy(qt_sb[p][:, st * QT:(st + 1) * QT], qps[st][:])
                nc.vector.tensor_copy(kt_sb[p][:, st * QT:(st + 1) * QT], kps[st][:])

        def proj_qk_chunk(p, st, pool):
            """One q + one k projection chunk for pair p, s-tile st (DVE evac:
            runs interleaved with attention where ACT is exp-bound)."""
            qp = pool.tile([128, QT], F32, name="qp", tag="qkseq")
            for i in range(DC):
                nc.tensor.matmul(
                    qp[:], wq_sb[i][:, p * 128:(p + 1) * 128],
                    xts[i][:, st * QT:(st + 1) * QT],
                    start=(i == 0), stop=(i == DC - 1))
            nc.vector.tensor_copy(qt_sb[p][:, st * QT:(st + 1) * QT], qp[:])
            kp = pool.tile([128, QT], F32, name="kp", tag="qkseq")
            for i in range(DC):
                nc.tensor.matmul(
                    kp[:], wk_sb[i][:, p * 128:(p + 1) * 128],
                    xts[i][:, st * QT:(st + 1) * QT],
                    start=(i == 0), stop=(i == DC - 1))
            nc.vector.tensor_copy(kt_sb[p][:, st * QT:(st + 1) * QT], kp[:])

        def attention(p, h, qt_i, scps, ctxps, att, attsm):
            """scores->exp->mask->PV for one (pair, head, q-tile). Returns the
            cps PSUM tile (rows 0:DH = unnormalized ctx, row DH = denom l);
            the caller normalizes (batched across both heads of the pair)."""
            hl = 2 * p + h
            r0, r1 = h * 64, h * 64 + 64
            q0 = qt_i * QT
            nkb = 4 * (qt_i + 1)
            # rows 0:DH ctx, row DH = l, rows DH+1:DH+33 scratch for the
            # l-row transpose read (garbage, never written)
            cps = ctxps.tile([DH + 33, QT], F32, name="cps", tag="cps")
            for g0 in range(0, nkb, 2):
                sp = scps.tile([128, 2 * QT], F32, name="sp", tag="sp")
                straddle2 = (g0 == 4 * qt_i + 2) and "masks" not in ABLATE \
                    and "no_straddle2" not in VARIANTS
                if straddle2:
                    # d=2 block full width; d=3 block only its live+diag 128
                    # cols, packed at sp[:, QT:QT+KB]
                    nc.tensor.matmul(
                        sp[:, 0:QT],
                        kt_sb[p][r0:r1, g0 * KB:(g0 + 1) * KB],
                        qt_sb[p][r0:r1, q0:q0 + QT],
                        start=True, stop=True)
                    nc.tensor.matmul(
                        sp[:, QT:QT + KB],
                        kt_sb[p][r0:r1, (g0 + 1) * KB:(g0 + 2) * KB],
                        qt_sb[p][r0:r1, q0 + 3 * KB:q0 + 4 * KB],
                        start=True, stop=True)
                else:
                    for u in range(2):
                        kb = g0 + u
                        nc.tensor.matmul(
                            sp[:, u * QT:(u + 1) * QT],
                            kt_sb[p][r0:r1, kb * KB:(kb + 1) * KB],
                            qt_sb[p][r0:r1, q0:q0 + QT],
                            start=True, stop=True)
                pt = att.tile([128, 2 * QT], BF, name="pt", tag="pt")
                if straddle2:
                    # exp only the live region [2KB:QT] of d=2 + packed d=3
                    nc.scalar.activation(
                        pt[:, 2 * KB:QT + KB], sp[:, 2 * KB:QT + KB],
                        mybir.ActivationFunctionType.Exp, scale=float(SCALE))
                    nc.gpsimd.memset(pt[:, 0:2 * KB], 0.0)
                    for off in (2 * KB, QT):
                        nc.vector.tensor_mul(
                            pt[:, off:off + KB], pt[:, off:off + KB], tri_sb[:])
                else:
                    nc.scalar.activation(
                        pt[:], sp[:], mybir.ActivationFunctionType.Exp,
                        scale=float(SCALE))
                    # causal masking for the d=0,1 straddle: memset the dead
                    # rectangle, multiply the [128,128] diagonal triangles
                    if "masks" in ABLATE:
                        pass
                    elif g0 == 4 * qt_i:          # blocks d=0, d=1
                        nc.gpsimd.memset(pt[:, QT:QT + KB], 0.0)
                        for off in (0, QT + KB):
                            nc.vector.tensor_mul(
                                pt[:, off:off + KB], pt[:, off:off + KB], tri_sb[:])
                    elif g0 == 4 * qt_i + 2:      # no_straddle2 fallback
                        nc.gpsimd.memset(pt[:, 0:2 * KB], 0.0)
                        nc.gpsimd.memset(pt[:, QT:QT + 3 * KB], 0.0)
                        for off in (2 * KB, QT + 3 * KB):
                            nc.vector.tensor_mul(
                                pt[:, off:off + KB], pt[:, off:off + KB], tri_sb[:])
                if straddle2:
                    # d=3 narrow PV first (no stop), then d=2 full width with
                    # stop=True so the accumulation group closes full-width
                    nc.tensor.matmul(
                        cps[0:DH + 1, 3 * KB:QT],
                        vt_sb[g0 + 1][:, hl * (DH + 1):(hl + 1) * (DH + 1)],
                        pt[:, QT:QT + KB],
                        start=False, stop=False)
                    nc.tensor.matmul(
                        cps[0:DH + 1, :],
                        vt_sb[g0][:, hl * (DH + 1):(hl + 1) * (DH + 1)],
                        pt[:, 0:QT],
                        start=(g0 == 0), stop=(g0 + 1 == nkb - 1))
                else:
                    for u in range(2):
                        kb = g0 + u
                        nc.tensor.matmul(
                            cps[0:DH + 1, :],
                            vt_sb[kb][:, hl * (DH + 1):(hl + 1) * (DH + 1)],
                            pt[:, u * QT:(u + 1) * QT],
                            start=(kb == 0), stop=(kb == nkb - 1))
            # normalize: 1/l via transposed layout so the DVE's iterative
            # divide sees a free dim of 16 strided elements, not 512:
            # StreamTranspose the 32-row band holding the l row, recip the
            # strided column, transpose back, broadcast, multiply.
            tt = attsm.tile([32, QT], F32, name="tt", tag="tt")
            nc.vector.transpose(tt[:], cps[DH:DH + 32, :])
            rec = attsm.tile([32, QT], F32, name="rec", tag="rec")
            tv = tt.rearrange("p (b c) -> p b c", c=32)
            rv = rec.rearrange("p (b c) -> p b c", c=32)
            if "recip" in ABLATE:
                nc.vector.tensor_copy(rv[:, :, 0:1], tv[:, :, 0:1])
            else:
                nc.vector.reciprocal(rv[:, :, 0:1], tv[:, :, 0:1])
            rrow = attsm.tile([32, QT], F32, name="rrow", tag="rrow")
            nc.vector.transpose(rrow[:], rec[:])
            rb = attsm.tile([64, QT], F32, name="rb", tag="rb")
            nc.gpsimd.partition_broadcast(rb[:], rrow[0:1, :])
            nc.vector.tensor_mul(
                ctx_sb[p][r0:r1, q0:q0 + QT], cps[0:DH, :], rb[:])
            return cps

        def outproj(qt_i, ph3ps, ph3sb):
            """partial out-projection rows for one q tile; bias added during
            the PSUM->SBUF evacuation (DVE), output bf16, then DMA out."""
            for qb in range(qt_i * 4, qt_i * 4 + 4):
                os_ = ph3sb.tile([128, D], BF, name="os", tag="os")
                for nh in range(2):
                    op = ph3ps.tile([128, 512], F32, name="op", tag="op")
                    for p in range(PAIRS):
                        nc.tensor.matmul(
                            op[:], ctx_sb[p][:, qb * 128:(qb + 1) * 128],
                            wo_sb[p][:, nh * 512:(nh + 1) * 512],
                            start=(p == 0), stop=(p == PAIRS - 1))
                    if "outio" in ABLATE:
                        continue
                    if "no_biasadd" in VARIANTS:
                        nc.vector.tensor_copy(os_[:, nh * 512:(nh + 1) * 512], op[:])
                    else:
                        nc.vector.tensor_add(
                            os_[:, nh * 512:(nh + 1) * 512], op[:],
                            bo_bc[:, nh * 512:(nh + 1) * 512])
                if "outdma" not in ABLATE and "outio" not in ABLATE:
                    nc.sync.dma_start(out[qb * 128:(qb + 1) * 128, :], os_[:])

        # phase A: q/k pair 0, chunk-pipelined against the input DMAs
        with tc.tile_pool(name="qk0ps", bufs=1, space="PSUM") as qk0ps:
            proj_qk_chunked(0, qk0ps)

        # phase B: V projection (own scope, closes before attention)
        skip_attn = "attn" in ABLATE
        with tc.tile_pool(name="vps", bufs=2, space="PSUM") as vps:
            for j in range(NKB):
                vp = vps.tile([128, HPC * DH], F32, name="vp", tag="vp")
                for i in range(DC):
                    nc.tensor.matmul(
                        vp[:], xts[i][:, j * 128:(j + 1) * 128], wv_sb[i][:],
                        start=(i == 0), stop=(i == DC - 1))
                vt_view = vt_sb[j].rearrange("p (h e) -> p h e", h=HPC)
                nc.vector.tensor_copy(
                    vt_view[:, :, 0:DH], vp.rearrange("p (h e) -> p h e", h=HPC))
                nc.gpsimd.memset(vt_view[:, :, DH:DH + 1], 1.0)

        # phase C: pair-0 attention with pair-1 q/k proj chunks interleaved
        with tc.tile_pool(name="att", bufs=6) as att, \
             tc.tile_pool(name="attsm", bufs=4) as attsm, \
             tc.tile_pool(name="scps", bufs=2, space="PSUM") as scps, \
             tc.tile_pool(name="ctxps", bufs=2, space="PSUM") as ctxps:

            with tc.tile_pool(name="qk1ps", bufs=2, space="PSUM") as qk1ps:
                for qt_i in range(NQT):
                    for h in range(2):
                        if not skip_attn:
                            attention(0, h, qt_i, scps, ctxps, att, attsm)
                    proj_qk_chunk(1, qt_i, qk1ps)

            # phase D: pair-1 attention, out-projection interleaved per
            # finished q tile
            with tc.tile_pool(name="ph3ps", bufs=2, space="PSUM") as ph3ps, \
                 tc.tile_pool(name="ph3sb", bufs=3) as ph3sb:
                for qt_i in range(NQT):
                    for h in range(2):
                        if not skip_attn:
                            attention(1, h, qt_i, scps, ctxps, att, attsm)
                    if "outproj" not in ABLATE and not skip_attn and qt_i > 0:
                        outproj(qt_i - 1, ph3ps, ph3sb)
                if "outproj" not in ABLATE and not skip_attn:
                    outproj(NQT - 1, ph3ps, ph3sb)

    nc.compile()
    return nc


_NC = None
PROFILE = False
TRACE_CORES = (0,)
LAST_RESULT = None


def _get_nc():
    global _NC
    if _NC is None:
        _NC = _build()
    return _NC


def kernel(x, Wq, Wk, Wv, Wo, bo):
    x = np.asarray(x, dtype=np.float32)
    Wq = np.asarray(Wq, dtype=np.float32)
    Wk = np.asarray(Wk, dtype=np.float32)
    Wv = np.asarray(Wv, dtype=np.float32)
    Wo = np.asarray(Wo, dtype=np.float32)
    bo = np.asarray(bo, dtype=np.float32)

    nc = _get_nc()

    in_maps = _prepare_in_maps(x, Wq, Wk, Wv, Wo, bo)

    global LAST_RESULT
    kw = {}
    if PROFILE:
        kw = dict(trace=True, trace_cores=list(TRACE_CORES))
    res = run_bass_kernel_spmd(nc, in_maps, core_ids=list(range(NCORES)), **kw)
    LAST_RESULT = res

    out = np.zeros((B, S, D), np.float32)
    for c in range(NCORES):
        b = c // 4
        out[b] += res.results[c]["out"].astype(np.float32)
    return out


def _prepare_in_maps(x, Wq, Wk, Wv, Wo, bo):
    kk = np.arange(KB)[:, None]
    qq = np.arange(KB)[None, :]
    import ml_dtypes
    tri = (kk <= qq).astype(ml_dtypes.bfloat16)

    bf16 = ml_dtypes.bfloat16
    xTs = [np.ascontiguousarray(x[b].T).astype(bf16) for b in range(B)]
    bo_row = np.ascontiguousarray(bo[None, :]).astype(np.float32)
    zeros_row = np.zeros((1, D), np.float32)

    in_maps = []
    for c in range(NCORES):
        b, g = divmod(c, 4)
        cs = slice(g * HPC * DH, (g + 1) * HPC * DH)
        in_maps.append({
            "xT": xTs[b],
            "wq": np.ascontiguousarray(Wq[:, cs]).astype(bf16),
            "wk": np.ascontiguousarray(Wk[:, cs]).astype(bf16),
            "wv": np.ascontiguousarray(Wv[:, cs]).astype(bf16),
            "wo": np.ascontiguousarray(Wo[cs, :]).astype(bf16),
            "bo_r": bo_row if g == 0 else zeros_row,
            "tri": tri,
        })
    return in_maps
